# revision 35
# baseline (speedup 1.0000x reference)
"""BarrierNet (MLP heads + dCBF closed-form QP) Trainium2 Bass kernel.

Data-parallel over 8 NeuronCores: batch 262144 is split into 8 shards of
32768 rows; the tiny MLP weights are replicated (folded with mean/std on
host) and each core computes its full shard independently. No collectives.

v2 design (vs f32r baseline at 133 us):
  * All matmuls in bf16: f32r's fp32_mode=HIGH kept the PE HAM clock
    duty-cycled at 0.65-1.2 GHz; bf16 streams 1 col/cycle at 2.4 GHz
    with FWL weight loads.
  * x for the matmul path is staged as [32, 8192] bf16 (4 row-groups of
    8192 rows; matmul operand partition bases must be 32-aligned so
    feature f of group g sits on SBUF partition 32g+f), DMA'd in column
    slices that cover all 4 groups so compute starts on the first slice.
  * Per 1024-row iter: 2x L1 (K=8, tile_position row strips 32g so the
    pair runs concurrently in disjoint 32-row PE tiles), 2x L2 (K=128),
    2x L3 head accumulation via a sliding window of a zero-padded
    weight tensor (chunk jh lands on PSUM partitions {32v + jh}); both
    halves share one head PSUM bank.
  * PSUM: h [128,1024]x2 + m [128,512]x3 + head [128,512]x1 = 8 banks.
  * PSUM->SBUF relu drains are rotated across Scalar/Vector/GpSimd so
    no single engine bottlenecks; QP dCBF math runs on [128,128]
    batch-major tiles overlapping the other half's matmuls.
"""

import os
import sys

import numpy as np

sys.path.insert(0, "/opt/trn_rl_repo")

import concourse.bass as bass
import concourse.tile as tile
from concourse import mybir
from concourse.bass_utils import run_bass_kernel_spmd

F32 = mybir.dt.float32
BF16 = mybir.dt.bfloat16
AF = mybir.ActivationFunctionType
ALU = mybir.AluOpType

B = 262144
NF = 8
NCORES = 8
BC = B // NCORES   # 32768 rows per core
NG = 4             # row groups per core (operand bases must be 32-aligned)
GR = BC // NG      # 8192 rows per group
CH = 512           # chunk columns (one PSUM bank of fp32)
NSLOT = GR // CH   # 16 chunks per group
HB = BC // 2       # 16384 rows per half
HQ = HB // 128     # 128: per-half batch-major free width
# x_tr column-slice sizes: small first slices for an early compute start
XSLICES = (256, 1792, 2048, 2048, 2048)
OBS_X, OBS_Y, RAD = 4.0, 6.0, 1.5
PI = float(np.pi)

N_WARMUP_MM = 20


def _build_program(std4, mean4, split_waits=True, reps=1):
    nc = bass.Bass()

    x_bm = nc.dram_tensor("x_bm", [BC, NF], F32, kind="ExternalInput")
    x_tr = nc.dram_tensor("x_tr", [32, GR], BF16, kind="ExternalInput")
    w1g = nc.dram_tensor("w1g", [128, 128], BF16, kind="ExternalInput")
    wmw = nc.dram_tensor("wmw", [128, 288], BF16, kind="ExternalInput")
    bias3 = nc.dram_tensor("bias3", [128, 3], F32, kind="ExternalInput")
    u_out = nc.dram_tensor("u", [BC, 2], F32, kind="ExternalOutput")

    with tile.TileContext(nc) as tc:
        from contextlib import ExitStack

        with ExitStack() as ctx:
            _body(ctx, tc, x_bm, x_tr, w1g, wmw, bias3, u_out,
                  std4, mean4, reps)
    if split_waits:
        _split_multi_waits(nc)
    return nc


def _split_multi_waits(nc):
    """walrus (this build) accepts at most one sync-wait per instruction;
    merge same-semaphore waits to their max threshold, then hoist any
    remaining extra waits onto standalone same-engine EventSemaphore ops."""
    for blk in nc.main_func.blocks:
        out = []
        for ins in blk.instructions:
            si = ins.sync_info
            waits = list(si.on_wait) if si is not None else []
            if len(waits) > 1:
                merged = {}
                for w in waits:
                    key = (w.sync_type, w.id)
                    prev = merged.get(key)
                    if (prev is None or (w.wait_value or 0) >
                            (prev.wait_value or 0)):
                        merged[key] = w
                waits = list(merged.values())
                if len(waits) == 1:
                    ins.sync_info = type(si)(on_wait=waits,
                                             on_update=list(si.on_update))
            if len(waits) > 1:
                for k, w in enumerate(waits[:-1]):
                    ev = mybir.InstEventSemaphore(
                        name=f"{ins.name}w{k}", ins=[], outs=[])
                    ev.engine = ins.engine
                    ev.sync_info = type(si)(on_wait=[w], on_update=[])
                    out.append(ev)
                ins.sync_info = type(si)(on_wait=[waits[-1]],
                                         on_update=list(si.on_update))
            out.append(ins)
        blk.instructions = out


def _relu_copy(nc, eng, dst, src, bias):
    """dst = relu(src + bias), PSUM->SBUF, on the given engine."""
    if eng is nc.scalar:
        nc.scalar.activation(dst, src, AF.Relu, bias=bias, scale=1.0)
    else:
        eng.tensor_scalar(dst, src, bias, 0.0, ALU.add, ALU.max)


def _body(ctx, tc, x_bm, x_tr, w1g, wmw, bias3, u_out,
          std4, mean4, reps):
    nc = tc.nc

    const = ctx.enter_context(tc.tile_pool(name="const", bufs=1))
    xtp = ctx.enter_context(tc.tile_pool(name="xtp", bufs=1))
    hp = ctx.enter_context(tc.tile_pool(name="hp", bufs=3))
    mp = ctx.enter_context(tc.tile_pool(name="mp", bufs=4))
    hs = ctx.enter_context(tc.tile_pool(name="hs", bufs=1))
    qp = ctx.enter_context(tc.tile_pool(name="qp", bufs=1))
    ps_h = ctx.enter_context(tc.tile_pool(name="ps_h", bufs=2, space="PSUM"))
    ps_m = ctx.enter_context(tc.tile_pool(name="ps_m", bufs=3, space="PSUM"))
    ps_hd = ctx.enter_context(tc.tile_pool(name="ps_hd", bufs=1, space="PSUM"))

    # ---- input + weight DMAs, ordered for earliest compute start ----
    w1g_sb = const.tile([128, 128], BF16)
    wmw_sb = const.tile([128, 288], BF16)
    bias3_sb = const.tile([128, 3], F32)
    # feature f of group g on partition 32g+f (32-aligned operand bases)
    xt_sb = xtp.tile([128, GR], BF16, name="xt_sb", tag="xt_sb")
    # batch-major x for the dCBF math: col h*1024 + c*8 + f is feature f
    # of row half*HB + p*128 + c
    xh_sb = xtp.tile([128, 2 * HQ * NF], F32, name="xh_sb", tag="xh_sb")

    # sync (HWDGE, earliest): w1g rows for warmups + half-0 groups
    # first (tiny), bias3, xt slice 0, wmw; half-1 w1g rows follow
    nc.sync.dma_start(out=w1g_sb[0:40, :], in_=w1g[0:40, :])
    nc.sync.dma_start(out=bias3_sb, in_=bias3[:, :])
    c0 = 0
    for si, w in enumerate(XSLICES):
        eng = nc.sync if si % 2 == 0 else nc.gpsimd
        for g in range(NG):
            eng.dma_start(out=xt_sb[32 * g:32 * g + NF, c0:c0 + w],
                          in_=x_tr[NF * g:NF * (g + 1), c0:c0 + w])
        if si == 0:
            nc.sync.dma_start(out=wmw_sb, in_=wmw[:, :])
            nc.sync.dma_start(out=w1g_sb[64:104, :], in_=w1g[64:104, :])
        c0 += w
    # the scalar queue carries only x_bm so the QP x tiles land early
    for h in range(2):
        src3h = x_bm[h * HB:(h + 1) * HB, :].rearrange(
            "(p c) f -> p c f", p=128)
        nc.scalar.dma_start(
            out=xh_sb[:, h * 1024:(h + 1) * 1024]
                .rearrange("p (c f) -> p c f", f=NF),
            in_=src3h,
        )

    wmt_sb = wmw_sb[:, 0:128]
    wz_sb = wmw_sb[:, 128:288]
    b1_sb = bias3_sb[:, 0:1]
    bm_sb = bias3_sb[:, 1:2]
    bh_sb = bias3_sb[:, 2:3]

    for _ in range(reps):
        _body_rep(nc, tc, const, xtp, hp, mp, hs, qp, ps_h, ps_m, ps_hd,
                  u_out, w1g_sb, wmt_sb, wz_sb, b1_sb, bm_sb,
                  bh_sb, xt_sb, xh_sb, std4, mean4)


def _qp_pre(nc, qp, half, xh_sb, std4, mean4):
    """x-only dCBF terms for one half (r_half = p*128 + c). Runs during
    the matmul phase. Returns the tile dict for _qp_post."""
    s0, s1c, s2c, s3 = std4
    m0, m1c, m2c, m3 = mean4

    def t(name):
        nm = f"{name}_{half}"
        return qp.tile([128, HQ], F32, name=nm, tag=nm)

    xs3 = xh_sb[:, half * 1024:(half + 1) * 1024].rearrange(
        "p (c f) -> p c f", f=NF)
    X0, X1, X2, X3 = (xs3[:, :, i] for i in range(4))

    ST, CT, DX, DY, V = t("ST"), t("CT"), t("DX"), t("DY"), t("V")

    def wrapped_sin(out, phase_bias, nm):
        # range-reduce to [-pi, pi] on DVE (Pool TS is ~2us/op; avoid)
        msk = t(f"mk{nm}")
        ph = t(f"ph{nm}")
        ph2 = t(f"p2{nm}")
        if s2c == 1.0 and phase_bias == 0.0:
            ph0 = X2
        else:
            ph0 = t(f"p0{nm}")
            nc.vector.tensor_scalar(ph0, X2, s2c, phase_bias,
                                    ALU.mult, ALU.add)
        nc.vector.tensor_scalar(msk, ph0, PI, None, ALU.is_gt)
        nc.vector.scalar_tensor_tensor(ph, msk, -2.0 * PI, ph0,
                                       ALU.mult, ALU.add)
        nc.vector.tensor_scalar(msk, ph, -PI, None, ALU.is_lt)
        nc.vector.scalar_tensor_tensor(ph2, msk, 2.0 * PI, ph,
                                       ALU.mult, ALU.add)
        nc.scalar.activation(out, ph2, AF.Sin)

    wrapped_sin(ST, m2c, "s")
    wrapped_sin(CT, m2c + PI / 2, "c")
    nc.vector.tensor_scalar(DX, X0, s0, m0 - OBS_X, ALU.mult, ALU.add)
    nc.vector.tensor_scalar(DY, X1, s1c, m1c - OBS_Y, ALU.mult, ALU.add)
    nc.vector.tensor_scalar(V, X3, s3, m3, ALU.mult, ALU.add)

    t1, t2, Aq, t3, t4, Bq = (t("t1"), t("t2"), t("Aq"), t("t3"), t("t4"),
                              t("Bq"))
    nc.gpsimd.tensor_tensor(t1, DX, CT, ALU.mult)
    nc.gpsimd.tensor_tensor(t2, DY, ST, ALU.mult)
    nc.gpsimd.tensor_tensor(Aq, t1, t2, ALU.add)       # A = dx ct + dy st
    nc.gpsimd.tensor_tensor(t3, DX, ST, ALU.mult)
    nc.gpsimd.tensor_tensor(t4, DY, CT, ALU.mult)
    nc.gpsimd.tensor_tensor(Bq, t3, t4, ALU.subtract)  # B = dx st - dy ct

    VB, VA = t("VB"), t("VA")
    nc.gpsimd.tensor_tensor(VB, V, Bq, ALU.mult)       # G1 = 2 VB
    nc.gpsimd.tensor_tensor(VA, V, Aq, ALU.mult)       # bdot = 2 VA

    DX2, DY2, BARp, V2d, VB2, A2 = (t("DX2"), t("DY2"), t("BARp"),
                                    t("V2d"), t("VB2"), t("A2"))
    nc.scalar.activation(DX2, DX, AF.Square)
    nc.scalar.activation(DY2, DY, AF.Square)
    nc.gpsimd.tensor_tensor(BARp, DX2, DY2, ALU.add)   # dx^2 + dy^2
    nc.scalar.activation(V2d, V, AF.Square, scale=float(np.sqrt(2.0)))
    nc.scalar.activation(VB2, VB, AF.Square, scale=2.0)  # G1^2
    nc.scalar.activation(A2, Aq, AF.Square, scale=2.0)   # G2^2

    GG, R = t("GG"), t("R")
    nc.vector.scalar_tensor_tensor(GG, VB2, 1e-12, A2, ALU.add, ALU.add)
    nc.vector.reciprocal(R, GG)
    return dict(Aq=Aq, VB=VB, VA=VA, BARp=BARp, V2d=V2d, R=R, t=t)


def _qp_post(nc, qp, half, pre, headsb, u_out, fast):
    """Head-dependent QP tail for one half. fast=True: DVE main chain
    with GpSimd side ops (for the end-of-kernel tail). fast=False: all
    on GpSimd (STT decomposed into TS+TT; GpSimd has no STT)."""
    t = pre["t"]
    Aq, VB, VA = pre["Aq"], pre["VB"], pre["VA"]
    BARp, V2d, R = pre["BARp"], pre["V2d"], pre["R"]
    if fast:
        ve, va = nc.vector, nc.gpsimd

        def STT(out, in0, sc, in1, op0, op1):
            nc.vector.scalar_tensor_tensor(out, in0, sc, in1, op0, op1)
    else:
        ve = va = nc.gpsimd
        cnt = [0]

        def STT(out, in0, sc, in1, op0, op1):
            tmp = t(f"sx{cnt[0]}")
            cnt[0] += 1
            nc.gpsimd.tensor_scalar(tmp, in0, sc, None, op0)
            nc.gpsimd.tensor_tensor(out, tmp, in1, op1)

    p1n, p2n, sg1, sg2 = t("p1n"), t("p2n"), t("sg1"), t("sg2")
    for v, dst in enumerate([p1n, p2n, sg1, sg2]):
        eng = (nc.sync, nc.gpsimd, nc.scalar, nc.sync)[v]
        eng.dma_start(
            out=dst,
            in_=headsb[32 * v:32 * v + 32, :].rearrange(
                "j (q c) -> j q c", q=4),
        )

    SS, SP, T5p, T4d = t("SS"), t("SP"), t("T5p"), t("T4d")
    ve.tensor_tensor(SS, sg1, sg2, ALU.add)
    va.tensor_tensor(SP, sg1, sg2, ALU.mult)
    STT(T5p, BARp, -RAD * RAD, SP, ALU.add, ALU.mult)
    STT(T4d, SS, 8.0, VA, ALU.mult, ALU.mult)

    T1d, T2d, T3d, q1, q2, NUMn = (t("T1d"), t("T2d"), t("T3d"),
                                   t("q1"), t("q2"), t("NUMn"))
    STT(T1d, VB, 2.0, p1n, ALU.mult, ALU.mult)
    STT(T2d, Aq, 2.0, p2n, ALU.mult, ALU.mult)
    ve.tensor_tensor(T3d, T1d, T2d, ALU.subtract)  # = -Gp
    ve.tensor_tensor(q1, T3d, V2d, ALU.subtract)
    ve.tensor_tensor(q2, q1, T4d, ALU.subtract)
    STT(NUMn, T5p, 16.0, q2, ALU.mult, ALU.subtract)  # = Gp + hcon

    L0, LAM2 = t("L0"), t("LAM2")
    ve.tensor_tensor(L0, NUMn, R, ALU.mult)
    ve.tensor_scalar(LAM2, L0, -2.0, 0.0, ALU.mult, ALU.max)  # 2 lam

    u_bm = qp.tile([128, 2 * HQ], F32, name=f"u_bm_{half}",
                   tag=f"u_bm_{half}")
    ub3 = u_bm[:].rearrange("p (c v) -> p c v", v=2)
    m1t, m2t = t("m1t"), t("m2t")
    ve.tensor_tensor(m1t, LAM2, VB, ALU.mult)
    ve.tensor_tensor(ub3[:, :, 0], p1n, m1t, ALU.subtract)
    va.tensor_tensor(m2t, LAM2, Aq, ALU.mult)
    va.tensor_tensor(ub3[:, :, 1], p2n, m2t, ALU.add)

    uo3 = u_out[half * HB:(half + 1) * HB, :].rearrange(
        "(p c) v -> p c v", p=128)
    nc.gpsimd.dma_start(out=uo3[:, 0:64, :], in_=ub3[:, 0:64, :])
    nc.sync.dma_start(out=uo3[:, 64:128, :], in_=ub3[:, 64:128, :])


def _body_rep(nc, tc, const, xtp, hp, mp, hs, qp, ps_h, ps_m, ps_hd,
              u_out, w1g_sb, wmt_sb, wz_sb, b1_sb, bm_sb, bh_sb,
              xt_sb, xh_sb, std4, mean4):
    # head accumulator: ONE bank shared by both halves; also the
    # PE-warmup dump target
    head_ps = ps_hd.tile([128, CH], F32, name="head", tag="head")

    # PE warmup: dummy matmuls keep the HAM clock ramping while the
    # input DMAs run (overwritten by the real accumulation's start=True)
    for _ in range(N_WARMUP_MM):
        nc.tensor.matmul(head_ps[:, 0:128], w1g_sb[0:8, :],
                         w1g_sb[0:8, 0:128], start=True, stop=True)

    qp_pre = [None, None]
    mi = 0   # m-drain engine rotation counter
    hi = 0   # h-drain second-half rotation counter

    for half in range(2):
        for slot in range(NSLOT):
            # ---- L1: the half's two groups, disjoint 32-row PE strips
            h_ps = ps_h.tile([128, 2 * CH], F32, name="h_ps", tag="h_ps")
            for k in range(2):
                g = 2 * half + k
                nc.tensor.matmul(
                    h_ps[:, k * CH:(k + 1) * CH],
                    w1g_sb[32 * g:32 * g + 8, :],
                    xt_sb[32 * g:32 * g + 8, slot * CH:(slot + 1) * CH],
                    start=True, stop=True,
                    tile_position=(32 * g, 0),
                )
            # GPSIMD cannot read PSUM: drains go to Scalar + Vector only.
            # Alternate: one engine takes h [128,1024], the other takes
            # the two m [128,512] chunks this iter.
            eng_h = nc.scalar if hi % 2 == 0 else nc.vector
            eng_m2 = nc.vector if hi % 2 == 0 else nc.scalar
            hi += 1
            h_sb = hp.tile([128, 2 * CH], BF16, name="h_sb", tag="h_sb")
            _relu_copy(nc, eng_h, h_sb, h_ps, b1_sb)

            for k in range(2):
                jh = k * NSLOT + slot    # chunk index within half
                step = slot * 2 + k      # head accumulation step

                m_ps = ps_m.tile([128, CH], F32, name="m_ps", tag="m_ps")
                nc.tensor.matmul(
                    m_ps, wmt_sb, h_sb[:, k * CH:(k + 1) * CH],
                    start=True, stop=True)
                m_sb = mp.tile([128, CH], BF16, name="m_sb", tag="m_sb")
                mi += 1
                _relu_copy(nc, eng_m2, m_sb, m_ps, bm_sb)

                nc.tensor.matmul(
                    head_ps,
                    wz_sb[:, 31 - jh:159 - jh],
                    m_sb,
                    start=(step == 0), stop=(step == 31),
                )

            if half == 0 and slot == 5:
                qp_pre[0] = _qp_pre(nc, qp, 0, xh_sb, std4, mean4)
            if half == 0 and slot == 15:
                qp_pre[1] = _qp_pre(nc, qp, 1, xh_sb, std4, mean4)

        # drain this half's heads to SBUF, then run the QP tail
        hsb = hs.tile([128, CH], F32, name=f"hsb{half}", tag=f"hsb{half}")
        nc.vector.tensor_scalar(hsb[0:64, :], head_ps[0:64, :],
                                -1.0, bh_sb[0:64, :], ALU.mult, ALU.add)
        nc.scalar.activation(hsb[64:128, :], head_ps[64:128, :],
                             AF.Sigmoid, bias=bh_sb[64:128, :], scale=1.0)
        _qp_post(nc, qp, half, qp_pre[half], hsb, u_out, fast=True)


def _host_prepare(inputs):
    """Fold mean/std into L1, build packed weight/bias tensors."""
    import ml_dtypes

    bf16 = ml_dtypes.bfloat16

    x = np.ascontiguousarray(inputs["x"], dtype=np.float32)
    mean = np.asarray(inputs["mean"], dtype=np.float32)
    std = np.asarray(inputs["std"], dtype=np.float32)
    W1 = np.asarray(inputs["W1"], dtype=np.float32)
    b1 = np.asarray(inputs["b1"], dtype=np.float32)
    W21 = np.asarray(inputs["W21"], dtype=np.float32)
    b21 = np.asarray(inputs["b21"], dtype=np.float32)
    W22 = np.asarray(inputs["W22"], dtype=np.float32)
    b22 = np.asarray(inputs["b22"], dtype=np.float32)
    W31 = np.asarray(inputs["W31"], dtype=np.float32)
    b31 = np.asarray(inputs["b31"], dtype=np.float32)
    W32 = np.asarray(inputs["W32"], dtype=np.float32)
    b32 = np.asarray(inputs["b32"], dtype=np.float32)

    W1eff = W1 * std[None, :]                      # [128, 8]
    b1eff = (b1 + W1 @ mean).astype(np.float32)    # [128]
    w1t = np.ascontiguousarray(W1eff.T)            # [8, 128]
    w1g = np.zeros((128, 128), np.float32)
    for g in range(NG):
        w1g[32 * g:32 * g + 8, :] = w1t
    w1g = w1g.astype(bf16)

    Wmid = np.vstack([W21, W22]).astype(np.float32)   # [128, 128]
    wmt = np.ascontiguousarray(Wmid.T)
    bmid = np.concatenate([b21, b22]).astype(np.float32)[:, None]

    Whead = np.zeros((4, 128), np.float32)
    Whead[0:2, 0:64] = W31
    Whead[2:4, 64:128] = W32
    wz = np.zeros((128, 160), np.float32)
    for v in range(4):
        wz[:, 31 + 32 * v] = Whead[v, :]

    bhead = np.zeros((128, 1), np.float32)
    bhead[0:32, 0] = -b31[0]
    bhead[32:64, 0] = -b31[1]
    bhead[64:96, 0] = b32[0]
    bhead[96:128, 0] = b32[1]

    std4 = tuple(float(std[i]) for i in range(4))
    mean4 = tuple(float(mean[i]) for i in range(4))

    wmw = np.ascontiguousarray(
        np.concatenate([wmt, wz], axis=1)).astype(bf16)
    bias3 = np.ascontiguousarray(
        np.concatenate([b1eff[:, None], bmid, bhead], axis=1))

    common = {"w1g": w1g, "wmw": wmw, "bias3": bias3}

    in_maps = []
    for c in range(NCORES):
        xs = x[c * BC:(c + 1) * BC]               # [32768, 8]
        # transposed / grouped layout: row 8g+f = feature f of group g
        xtr = np.ascontiguousarray(
            xs.reshape(NG, GR, NF).transpose(0, 2, 1).reshape(
                32, GR)).astype(bf16)
        in_maps.append({"x_bm": xs, "x_tr": xtr, **common})
    return in_maps, std4, mean4


def kernel(**inputs):
    in_maps, std4, mean4 = _host_prepare(inputs)
    nc = _build_program(std4, mean4)
    last_err = None
    for attempt in range(3):
        try:
            res = run_bass_kernel_spmd(nc, in_maps, list(range(NCORES)))
            break
        except Exception as e:  # transient axon/NRT flakes
            last_err = e
            if attempt == 2:
                raise
            import time

            time.sleep(5)
    u = np.concatenate([res.results[c]["u"] for c in range(NCORES)], axis=0)
    return u.astype(np.float32)


if __name__ == "__main__":
    rng = np.random.default_rng(0)
    demo = {
        "x": rng.standard_normal((B, NF), dtype=np.float32),
        "mean": np.zeros(NF, np.float32),
        "std": np.ones(NF, np.float32),
        "W1": rng.standard_normal((128, NF), dtype=np.float32) * 0.3,
        "b1": rng.standard_normal(128, dtype=np.float32) * 0.3,
        "W21": rng.standard_normal((64, 128), dtype=np.float32) * 0.08,
        "b21": rng.standard_normal(64, dtype=np.float32) * 0.08,
        "W22": rng.standard_normal((64, 128), dtype=np.float32) * 0.08,
        "b22": rng.standard_normal(64, dtype=np.float32) * 0.08,
        "W31": rng.standard_normal((2, 64), dtype=np.float32) * 0.1,
        "b31": rng.standard_normal(2, dtype=np.float32) * 0.1,
        "W32": rng.standard_normal((2, 64), dtype=np.float32) * 0.1,
        "b32": rng.standard_normal(2, dtype=np.float32) * 0.1,
        "sgn": np.int64(1),
    }
    out = kernel(**demo)
    print(out.shape, out.dtype)


# revision 37
# speedup vs baseline: 1.0284x; 1.0284x over previous
"""BarrierNet (MLP heads + dCBF closed-form QP) Trainium2 Bass kernel.

Data-parallel over 8 NeuronCores: batch 262144 is split into 8 shards of
32768 rows; the tiny MLP weights are replicated (folded with mean/std on
host) and each core computes its full shard independently. No collectives.

v2 design (vs f32r baseline at 133 us):
  * All matmuls in bf16: f32r's fp32_mode=HIGH kept the PE HAM clock
    duty-cycled at 0.65-1.2 GHz; bf16 streams 1 col/cycle at 2.4 GHz
    with FWL weight loads.
  * x for the matmul path is staged as [32, 8192] bf16 (4 row-groups of
    8192 rows; matmul operand partition bases must be 32-aligned so
    feature f of group g sits on SBUF partition 32g+f), DMA'd in column
    slices that cover all 4 groups so compute starts on the first slice.
  * Per 1024-row iter: 2x L1 (K=8, tile_position row strips 32g so the
    pair runs concurrently in disjoint 32-row PE tiles), 2x L2 (K=128),
    2x L3 head accumulation via a sliding window of a zero-padded
    weight tensor (chunk jh lands on PSUM partitions {32v + jh}); both
    halves share one head PSUM bank.
  * PSUM: h [128,1024]x2 + m [128,512]x3 + head [128,512]x1 = 8 banks.
  * PSUM->SBUF relu drains alternate between ScalarE and VectorE (GpSimd
    cannot access PSUM): per iter one engine takes the h [128,1024]
    drain, the other takes the two m [128,512] drains.
  * QP dCBF math runs on [128,128] batch-major f32 tiles overlapping the
    matmul phase: sin range-reduction/STT chains on VectorE (native STT;
    GpSimd tensor_scalar measures ~2us/op - avoid), plain tensor_tensor
    ops on GpSimd, sin/square/sigmoid on ScalarE. Both halves' sins are
    traced before the first sigmoid so the act-table loads twice total.
  * Engine FIFOs are strict in-order: any QP op that waits on a late
    input blocks every drain queued behind it and stalls the PE (HAM
    then re-throttles the clock to 1.2 GHz). The x_bm load therefore
    owns the scalar DMA queue, and qp_pre is traced only at points
    where its inputs have already landed.
"""

import sys

import numpy as np

sys.path.insert(0, "/opt/trn_rl_repo")

import concourse.bass as bass
import concourse.tile as tile
from concourse import mybir
from concourse.bass_utils import run_bass_kernel_spmd

F32 = mybir.dt.float32
BF16 = mybir.dt.bfloat16
AF = mybir.ActivationFunctionType
ALU = mybir.AluOpType

B = 262144
NF = 8
NCORES = 8
BC = B // NCORES   # 32768 rows per core
NG = 4             # row groups per core (operand bases must be 32-aligned)
GR = BC // NG      # 8192 rows per group
CH = 512           # chunk columns (one PSUM bank of fp32)
NSLOT = GR // CH   # 16 chunks per group
HB = BC // 2       # 16384 rows per half
HQ = HB // 128     # 128: per-half batch-major free width
# x_tr column-slice sizes: small first slices for an early compute start
XSLICES = (512, 1536, 2048, 2048, 2048)
OBS_X, OBS_Y, RAD = 4.0, 6.0, 1.5
PI = float(np.pi)

N_WARMUP_MM = 20


def _build_program(std4, mean4, split_waits=True, reps=1):
    nc = bass.Bass()

    x_bm = nc.dram_tensor("x_bm", [BC, NF], F32, kind="ExternalInput")
    x_tr = nc.dram_tensor("x_tr", [32, GR], BF16, kind="ExternalInput")
    w1g = nc.dram_tensor("w1g", [128, 128], BF16, kind="ExternalInput")
    wmw = nc.dram_tensor("wmw", [128, 288], BF16, kind="ExternalInput")
    bias3 = nc.dram_tensor("bias3", [128, 3], F32, kind="ExternalInput")
    u_out = nc.dram_tensor("u", [BC, 2], F32, kind="ExternalOutput")

    with tile.TileContext(nc) as tc:
        from contextlib import ExitStack

        with ExitStack() as ctx:
            _body(ctx, tc, x_bm, x_tr, w1g, wmw, bias3, u_out,
                  std4, mean4, reps)
    if split_waits:
        _split_multi_waits(nc)
    return nc


def _split_multi_waits(nc):
    """walrus (this build) accepts at most one sync-wait per instruction;
    merge same-semaphore waits to their max threshold, then hoist any
    remaining extra waits onto standalone same-engine EventSemaphore ops."""
    for blk in nc.main_func.blocks:
        out = []
        for ins in blk.instructions:
            si = ins.sync_info
            waits = list(si.on_wait) if si is not None else []
            if len(waits) > 1:
                merged = {}
                for w in waits:
                    key = (w.sync_type, w.id)
                    prev = merged.get(key)
                    if (prev is None or (w.wait_value or 0) >
                            (prev.wait_value or 0)):
                        merged[key] = w
                waits = list(merged.values())
                if len(waits) == 1:
                    ins.sync_info = type(si)(on_wait=waits,
                                             on_update=list(si.on_update))
            if len(waits) > 1:
                for k, w in enumerate(waits[:-1]):
                    ev = mybir.InstEventSemaphore(
                        name=f"{ins.name}w{k}", ins=[], outs=[])
                    ev.engine = ins.engine
                    ev.sync_info = type(si)(on_wait=[w], on_update=[])
                    out.append(ev)
                ins.sync_info = type(si)(on_wait=[waits[-1]],
                                         on_update=list(si.on_update))
            out.append(ins)
        blk.instructions = out


def _relu_copy(nc, eng, dst, src, bias):
    """dst = relu(src + bias), PSUM->SBUF, on the given engine."""
    if eng is nc.scalar:
        nc.scalar.activation(dst, src, AF.Relu, bias=bias, scale=1.0)
    else:
        eng.tensor_scalar(dst, src, bias, 0.0, ALU.add, ALU.max)


def _body(ctx, tc, x_bm, x_tr, w1g, wmw, bias3, u_out,
          std4, mean4, reps):
    nc = tc.nc

    const = ctx.enter_context(tc.tile_pool(name="const", bufs=1))
    xtp = ctx.enter_context(tc.tile_pool(name="xtp", bufs=1))
    hp = ctx.enter_context(tc.tile_pool(name="hp", bufs=3))
    mp = ctx.enter_context(tc.tile_pool(name="mp", bufs=4))
    hs = ctx.enter_context(tc.tile_pool(name="hs", bufs=1))
    qp = ctx.enter_context(tc.tile_pool(name="qp", bufs=1))
    ps_h = ctx.enter_context(tc.tile_pool(name="ps_h", bufs=2, space="PSUM"))
    ps_m = ctx.enter_context(tc.tile_pool(name="ps_m", bufs=3, space="PSUM"))
    ps_hd = ctx.enter_context(tc.tile_pool(name="ps_hd", bufs=1, space="PSUM"))

    # ---- input + weight DMAs, ordered for earliest compute start ----
    w1g_sb = const.tile([128, 128], BF16)
    wmw_sb = const.tile([128, 288], BF16)
    bias3_sb = const.tile([128, 3], F32)
    # feature f of group g on partition 32g+f (32-aligned operand bases)
    xt_sb = xtp.tile([128, GR], BF16, name="xt_sb", tag="xt_sb")
    # batch-major x for the dCBF math: col h*1024 + c*8 + f is feature f
    # of row half*HB + p*128 + c
    xh_sb = xtp.tile([128, 2 * HQ * NF], F32, name="xh_sb", tag="xh_sb")

    # sync (HWDGE, earliest): w1g (gates L1+warmups), bias3 (gates the
    # first h-drain), xt slice 0, then wmw (gates L2)
    nc.sync.dma_start(out=w1g_sb, in_=w1g[:, :])
    nc.sync.dma_start(out=bias3_sb, in_=bias3[:, :])
    c0 = 0
    for si, w in enumerate(XSLICES):
        eng = nc.sync if si % 2 == 0 else nc.gpsimd
        for g in range(NG):
            eng.dma_start(out=xt_sb[32 * g:32 * g + NF, c0:c0 + w],
                          in_=x_tr[NF * g:NF * (g + 1), c0:c0 + w])
        if si == 0:
            nc.sync.dma_start(out=wmw_sb, in_=wmw[:, :])
        c0 += w
    # the scalar queue carries only x_bm so the QP x tiles land early
    for h in range(2):
        src3h = x_bm[h * HB:(h + 1) * HB, :].rearrange(
            "(p c) f -> p c f", p=128)
        nc.scalar.dma_start(
            out=xh_sb[:, h * 1024:(h + 1) * 1024]
                .rearrange("p (c f) -> p c f", f=NF),
            in_=src3h,
        )

    wmt_sb = wmw_sb[:, 0:128]
    wz_sb = wmw_sb[:, 128:288]
    b1_sb = bias3_sb[:, 0:1]
    bm_sb = bias3_sb[:, 1:2]
    bh_sb = bias3_sb[:, 2:3]

    for _ in range(reps):
        _body_rep(nc, tc, const, xtp, hp, mp, hs, qp, ps_h, ps_m, ps_hd,
                  u_out, w1g_sb, wmt_sb, wz_sb, b1_sb, bm_sb,
                  bh_sb, xt_sb, xh_sb, std4, mean4)


def _qp_pre(nc, qp, half, xh_sb, std4, mean4):
    """x-only dCBF terms for one half (r_half = p*128 + c). Runs during
    the matmul phase. Returns the tile dict for _qp_post."""
    s0, s1c, s2c, s3 = std4
    m0, m1c, m2c, m3 = mean4

    def t(name):
        nm = f"{name}_{half}"
        return qp.tile([128, HQ], F32, name=nm, tag=nm)

    xs3 = xh_sb[:, half * 1024:(half + 1) * 1024].rearrange(
        "p (c f) -> p c f", f=NF)
    X0, X1, X2, X3 = (xs3[:, :, i] for i in range(4))

    ST, CT, DX, DY, V = t("ST"), t("CT"), t("DX"), t("DY"), t("V")

    def wrapped_sin(out, phase_bias, nm):
        # range-reduce to [-pi, pi] on DVE (Pool TS is ~2us/op; avoid)
        msk = t(f"mk{nm}")
        ph = t(f"ph{nm}")
        ph2 = t(f"p2{nm}")
        if s2c == 1.0 and phase_bias == 0.0:
            ph0 = X2
        else:
            ph0 = t(f"p0{nm}")
            nc.vector.tensor_scalar(ph0, X2, s2c, phase_bias,
                                    ALU.mult, ALU.add)
        nc.vector.tensor_scalar(msk, ph0, PI, None, ALU.is_gt)
        nc.vector.scalar_tensor_tensor(ph, msk, -2.0 * PI, ph0,
                                       ALU.mult, ALU.add)
        nc.vector.tensor_scalar(msk, ph, -PI, None, ALU.is_lt)
        nc.vector.scalar_tensor_tensor(ph2, msk, 2.0 * PI, ph,
                                       ALU.mult, ALU.add)
        nc.scalar.activation(out, ph2, AF.Sin)

    wrapped_sin(ST, m2c, "s")
    wrapped_sin(CT, m2c + PI / 2, "c")
    nc.vector.tensor_scalar(DX, X0, s0, m0 - OBS_X, ALU.mult, ALU.add)
    nc.vector.tensor_scalar(DY, X1, s1c, m1c - OBS_Y, ALU.mult, ALU.add)
    nc.vector.tensor_scalar(V, X3, s3, m3, ALU.mult, ALU.add)

    t1, t2, Aq, t3, t4, Bq = (t("t1"), t("t2"), t("Aq"), t("t3"), t("t4"),
                              t("Bq"))
    nc.gpsimd.tensor_tensor(t1, DX, CT, ALU.mult)
    nc.gpsimd.tensor_tensor(t2, DY, ST, ALU.mult)
    nc.gpsimd.tensor_tensor(Aq, t1, t2, ALU.add)       # A = dx ct + dy st
    nc.gpsimd.tensor_tensor(t3, DX, ST, ALU.mult)
    nc.gpsimd.tensor_tensor(t4, DY, CT, ALU.mult)
    nc.gpsimd.tensor_tensor(Bq, t3, t4, ALU.subtract)  # B = dx st - dy ct

    VB, VA = t("VB"), t("VA")
    nc.gpsimd.tensor_tensor(VB, V, Bq, ALU.mult)       # G1 = 2 VB
    nc.gpsimd.tensor_tensor(VA, V, Aq, ALU.mult)       # bdot = 2 VA

    DX2, DY2, BARp, V2d, VB2, A2 = (t("DX2"), t("DY2"), t("BARp"),
                                    t("V2d"), t("VB2"), t("A2"))
    nc.scalar.activation(DX2, DX, AF.Square)
    nc.scalar.activation(DY2, DY, AF.Square)
    nc.gpsimd.tensor_tensor(BARp, DX2, DY2, ALU.add)   # dx^2 + dy^2
    nc.scalar.activation(V2d, V, AF.Square, scale=float(np.sqrt(2.0)))
    nc.scalar.activation(VB2, VB, AF.Square, scale=2.0)  # G1^2
    nc.scalar.activation(A2, Aq, AF.Square, scale=2.0)   # G2^2

    GG, R = t("GG"), t("R")
    nc.vector.scalar_tensor_tensor(GG, VB2, 1e-12, A2, ALU.add, ALU.add)
    nc.vector.reciprocal(R, GG)
    return dict(Aq=Aq, VB=VB, VA=VA, BARp=BARp, V2d=V2d, R=R, t=t)


def _qp_post(nc, qp, half, pre, headsb, u_out, fast):
    """Head-dependent QP tail for one half. fast=True: DVE main chain
    with GpSimd side ops (for the end-of-kernel tail). fast=False: all
    on GpSimd (STT decomposed into TS+TT; GpSimd has no STT)."""
    t = pre["t"]
    Aq, VB, VA = pre["Aq"], pre["VB"], pre["VA"]
    BARp, V2d, R = pre["BARp"], pre["V2d"], pre["R"]
    if fast:
        ve, va = nc.vector, nc.gpsimd

        def STT(out, in0, sc, in1, op0, op1):
            nc.vector.scalar_tensor_tensor(out, in0, sc, in1, op0, op1)
    else:
        ve = va = nc.gpsimd
        cnt = [0]

        def STT(out, in0, sc, in1, op0, op1):
            tmp = t(f"sx{cnt[0]}")
            cnt[0] += 1
            nc.gpsimd.tensor_scalar(tmp, in0, sc, None, op0)
            nc.gpsimd.tensor_tensor(out, tmp, in1, op1)

    p1n, p2n, sg1, sg2 = t("p1n"), t("p2n"), t("sg1"), t("sg2")
    for v, dst in enumerate([p1n, p2n, sg1, sg2]):
        eng = (nc.sync, nc.gpsimd, nc.scalar, nc.sync)[v]
        eng.dma_start(
            out=dst,
            in_=headsb[32 * v:32 * v + 32, :].rearrange(
                "j (q c) -> j q c", q=4),
        )

    SS, SP, T5p, T4d = t("SS"), t("SP"), t("T5p"), t("T4d")
    ve.tensor_tensor(SS, sg1, sg2, ALU.add)
    va.tensor_tensor(SP, sg1, sg2, ALU.mult)
    STT(T5p, BARp, -RAD * RAD, SP, ALU.add, ALU.mult)
    STT(T4d, SS, 8.0, VA, ALU.mult, ALU.mult)

    T1d, T2d, T3d, q1, q2, NUMn = (t("T1d"), t("T2d"), t("T3d"),
                                   t("q1"), t("q2"), t("NUMn"))
    STT(T1d, VB, 2.0, p1n, ALU.mult, ALU.mult)
    STT(T2d, Aq, 2.0, p2n, ALU.mult, ALU.mult)
    ve.tensor_tensor(T3d, T1d, T2d, ALU.subtract)  # = -Gp
    ve.tensor_tensor(q1, T3d, V2d, ALU.subtract)
    ve.tensor_tensor(q2, q1, T4d, ALU.subtract)
    STT(NUMn, T5p, 16.0, q2, ALU.mult, ALU.subtract)  # = Gp + hcon

    L0, LAM2 = t("L0"), t("LAM2")
    ve.tensor_tensor(L0, NUMn, R, ALU.mult)
    ve.tensor_scalar(LAM2, L0, -2.0, 0.0, ALU.mult, ALU.max)  # 2 lam

    u_bm = qp.tile([128, 2 * HQ], F32, name=f"u_bm_{half}",
                   tag=f"u_bm_{half}")
    ub3 = u_bm[:].rearrange("p (c v) -> p c v", v=2)
    m1t, m2t = t("m1t"), t("m2t")
    ve.tensor_tensor(m1t, LAM2, VB, ALU.mult)
    ve.tensor_tensor(ub3[:, :, 0], p1n, m1t, ALU.subtract)
    va.tensor_tensor(m2t, LAM2, Aq, ALU.mult)
    va.tensor_tensor(ub3[:, :, 1], p2n, m2t, ALU.add)

    uo3 = u_out[half * HB:(half + 1) * HB, :].rearrange(
        "(p c) v -> p c v", p=128)
    nc.gpsimd.dma_start(out=uo3[:, 0:64, :], in_=ub3[:, 0:64, :])
    nc.sync.dma_start(out=uo3[:, 64:128, :], in_=ub3[:, 64:128, :])


def _body_rep(nc, tc, const, xtp, hp, mp, hs, qp, ps_h, ps_m, ps_hd,
              u_out, w1g_sb, wmt_sb, wz_sb, b1_sb, bm_sb, bh_sb,
              xt_sb, xh_sb, std4, mean4):
    # head accumulator: ONE bank shared by both halves; also the
    # PE-warmup dump target
    head_ps = ps_hd.tile([128, CH], F32, name="head", tag="head")

    # PE warmup: dummy matmuls keep the HAM clock ramping while the
    # input DMAs run (overwritten by the real accumulation's start=True)
    for _ in range(N_WARMUP_MM):
        nc.tensor.matmul(head_ps[:, 0:128], w1g_sb[0:8, :],
                         w1g_sb[0:8, 0:128], start=True, stop=True)

    qp_pre = [None, None]
    mi = 0   # m-drain engine rotation counter
    hi = 0   # h-drain second-half rotation counter

    for half in range(2):
        for slot in range(NSLOT):
            # ---- L1: the half's two groups, disjoint 32-row PE strips
            h_ps = ps_h.tile([128, 2 * CH], F32, name="h_ps", tag="h_ps")
            for k in range(2):
                g = 2 * half + k
                nc.tensor.matmul(
                    h_ps[:, k * CH:(k + 1) * CH],
                    w1g_sb[32 * g:32 * g + 8, :],
                    xt_sb[32 * g:32 * g + 8, slot * CH:(slot + 1) * CH],
                    start=True, stop=True,
                    tile_position=(32 * g, 0),
                )
            # GPSIMD cannot read PSUM: drains go to Scalar + Vector only.
            # Alternate: one engine takes h [128,1024], the other takes
            # the two m [128,512] chunks this iter.
            eng_h = nc.scalar if hi % 2 == 0 else nc.vector
            eng_m2 = nc.vector if hi % 2 == 0 else nc.scalar
            hi += 1
            h_sb = hp.tile([128, 2 * CH], BF16, name="h_sb", tag="h_sb")
            _relu_copy(nc, eng_h, h_sb, h_ps, b1_sb)

            for k in range(2):
                jh = k * NSLOT + slot    # chunk index within half
                step = slot * 2 + k      # head accumulation step

                m_ps = ps_m.tile([128, CH], F32, name="m_ps", tag="m_ps")
                nc.tensor.matmul(
                    m_ps, wmt_sb, h_sb[:, k * CH:(k + 1) * CH],
                    start=True, stop=True)
                m_sb = mp.tile([128, CH], BF16, name="m_sb", tag="m_sb")
                mi += 1
                _relu_copy(nc, eng_m2, m_sb, m_ps, bm_sb)

                nc.tensor.matmul(
                    head_ps,
                    wz_sb[:, 31 - jh:159 - jh],
                    m_sb,
                    start=(step == 0), stop=(step == 31),
                )

            if half == 0 and slot == 5:
                qp_pre[0] = _qp_pre(nc, qp, 0, xh_sb, std4, mean4)
            if half == 0 and slot == 15:
                qp_pre[1] = _qp_pre(nc, qp, 1, xh_sb, std4, mean4)

        # drain this half's heads to SBUF, then run the QP tail
        hsb = hs.tile([128, CH], F32, name=f"hsb{half}", tag=f"hsb{half}")
        nc.vector.tensor_scalar(hsb[0:64, :], head_ps[0:64, :],
                                -1.0, bh_sb[0:64, :], ALU.mult, ALU.add)
        nc.scalar.activation(hsb[64:128, :], head_ps[64:128, :],
                             AF.Sigmoid, bias=bh_sb[64:128, :], scale=1.0)
        _qp_post(nc, qp, half, qp_pre[half], hsb, u_out, fast=True)


def _host_prepare(inputs):
    """Fold mean/std into L1, build packed weight/bias tensors."""
    import ml_dtypes

    bf16 = ml_dtypes.bfloat16

    x = np.ascontiguousarray(inputs["x"], dtype=np.float32)
    mean = np.asarray(inputs["mean"], dtype=np.float32)
    std = np.asarray(inputs["std"], dtype=np.float32)
    W1 = np.asarray(inputs["W1"], dtype=np.float32)
    b1 = np.asarray(inputs["b1"], dtype=np.float32)
    W21 = np.asarray(inputs["W21"], dtype=np.float32)
    b21 = np.asarray(inputs["b21"], dtype=np.float32)
    W22 = np.asarray(inputs["W22"], dtype=np.float32)
    b22 = np.asarray(inputs["b22"], dtype=np.float32)
    W31 = np.asarray(inputs["W31"], dtype=np.float32)
    b31 = np.asarray(inputs["b31"], dtype=np.float32)
    W32 = np.asarray(inputs["W32"], dtype=np.float32)
    b32 = np.asarray(inputs["b32"], dtype=np.float32)

    W1eff = W1 * std[None, :]                      # [128, 8]
    b1eff = (b1 + W1 @ mean).astype(np.float32)    # [128]
    w1t = np.ascontiguousarray(W1eff.T)            # [8, 128]
    w1g = np.zeros((128, 128), np.float32)
    for g in range(NG):
        w1g[32 * g:32 * g + 8, :] = w1t
    w1g = w1g.astype(bf16)

    Wmid = np.vstack([W21, W22]).astype(np.float32)   # [128, 128]
    wmt = np.ascontiguousarray(Wmid.T)
    bmid = np.concatenate([b21, b22]).astype(np.float32)[:, None]

    Whead = np.zeros((4, 128), np.float32)
    Whead[0:2, 0:64] = W31
    Whead[2:4, 64:128] = W32
    wz = np.zeros((128, 160), np.float32)
    for v in range(4):
        wz[:, 31 + 32 * v] = Whead[v, :]

    bhead = np.zeros((128, 1), np.float32)
    bhead[0:32, 0] = -b31[0]
    bhead[32:64, 0] = -b31[1]
    bhead[64:96, 0] = b32[0]
    bhead[96:128, 0] = b32[1]

    std4 = tuple(float(std[i]) for i in range(4))
    mean4 = tuple(float(mean[i]) for i in range(4))

    wmw = np.ascontiguousarray(
        np.concatenate([wmt, wz], axis=1)).astype(bf16)
    bias3 = np.ascontiguousarray(
        np.concatenate([b1eff[:, None], bmid, bhead], axis=1))

    common = {"w1g": w1g, "wmw": wmw, "bias3": bias3}

    in_maps = []
    for c in range(NCORES):
        xs = x[c * BC:(c + 1) * BC]               # [32768, 8]
        # transposed / grouped layout: row 8g+f = feature f of group g
        xtr = np.ascontiguousarray(
            xs.reshape(NG, GR, NF).transpose(0, 2, 1).reshape(
                32, GR)).astype(bf16)
        in_maps.append({"x_bm": xs, "x_tr": xtr, **common})
    return in_maps, std4, mean4


def kernel(**inputs):
    in_maps, std4, mean4 = _host_prepare(inputs)
    nc = _build_program(std4, mean4)
    last_err = None
    for attempt in range(3):
        try:
            res = run_bass_kernel_spmd(nc, in_maps, list(range(NCORES)))
            break
        except Exception as e:  # transient axon/NRT flakes
            last_err = e
            if attempt == 2:
                raise
            import time

            time.sleep(5)
    u = np.concatenate([res.results[c]["u"] for c in range(NCORES)], axis=0)
    return u.astype(np.float32)


if __name__ == "__main__":
    rng = np.random.default_rng(0)
    demo = {
        "x": rng.standard_normal((B, NF), dtype=np.float32),
        "mean": np.zeros(NF, np.float32),
        "std": np.ones(NF, np.float32),
        "W1": rng.standard_normal((128, NF), dtype=np.float32) * 0.3,
        "b1": rng.standard_normal(128, dtype=np.float32) * 0.3,
        "W21": rng.standard_normal((64, 128), dtype=np.float32) * 0.08,
        "b21": rng.standard_normal(64, dtype=np.float32) * 0.08,
        "W22": rng.standard_normal((64, 128), dtype=np.float32) * 0.08,
        "b22": rng.standard_normal(64, dtype=np.float32) * 0.08,
        "W31": rng.standard_normal((2, 64), dtype=np.float32) * 0.1,
        "b31": rng.standard_normal(2, dtype=np.float32) * 0.1,
        "W32": rng.standard_normal((2, 64), dtype=np.float32) * 0.1,
        "b32": rng.standard_normal(2, dtype=np.float32) * 0.1,
        "sgn": np.int64(1),
    }
    out = kernel(**demo)
    print(out.shape, out.dtype)


# revision 39
# speedup vs baseline: 1.0478x; 1.0189x over previous
"""BarrierNet (MLP heads + dCBF closed-form QP) Trainium2 Bass kernel.

Data-parallel over 8 NeuronCores: batch 262144 is split into 8 shards of
32768 rows; the tiny MLP weights are replicated (folded with mean/std on
host) and each core computes its full shard independently. No collectives.

v2 design (vs f32r baseline at 133 us):
  * All matmuls in bf16: f32r's fp32_mode=HIGH kept the PE HAM clock
    duty-cycled at 0.65-1.2 GHz; bf16 streams 1 col/cycle at 2.4 GHz
    with FWL weight loads.
  * x for the matmul path is staged as [32, 8192] bf16 (4 row-groups of
    8192 rows; matmul operand partition bases must be 32-aligned so
    feature f of group g sits on SBUF partition 32g+f), DMA'd in column
    slices that cover all 4 groups so compute starts on the first slice.
  * Per 1024-row iter: 2x L1 (K=8, tile_position row strips 32g so the
    pair runs concurrently in disjoint 32-row PE tiles), 2x L2 (K=128),
    2x L3 head accumulation via a sliding window of a zero-padded
    weight tensor (chunk jh lands on PSUM partitions {32v + jh}); both
    halves share one head PSUM bank.
  * PSUM: h [128,1024]x2 + m [128,512]x3 + head [128,512]x1 = 8 banks.
  * PSUM->SBUF relu drains alternate between ScalarE and VectorE (GpSimd
    cannot access PSUM): per iter one engine takes the h [128,1024]
    drain, the other takes the two m [128,512] drains.
  * QP dCBF math runs on [128,128] batch-major f32 tiles overlapping the
    matmul phase: sin range-reduction/STT chains on VectorE (native STT;
    GpSimd tensor_scalar measures ~2us/op - avoid), plain tensor_tensor
    ops on GpSimd, sin/square/sigmoid on ScalarE. Both halves' sins are
    traced before the first sigmoid so the act-table loads twice total.
  * Engine FIFOs are strict in-order: any QP op that waits on a late
    input blocks every drain queued behind it and stalls the PE (HAM
    then re-throttles the clock to 1.2 GHz). The x_bm load therefore
    owns the scalar DMA queue, and qp_pre is traced only at points
    where its inputs have already landed.
"""

import sys

import numpy as np

sys.path.insert(0, "/opt/trn_rl_repo")

import concourse.bass as bass
import concourse.tile as tile
from concourse import mybir
from concourse.bass_utils import run_bass_kernel_spmd

F32 = mybir.dt.float32
BF16 = mybir.dt.bfloat16
AF = mybir.ActivationFunctionType
ALU = mybir.AluOpType

B = 262144
NF = 8
NCORES = 8
BC = B // NCORES   # 32768 rows per core
NG = 4             # row groups per core (operand bases must be 32-aligned)
GR = BC // NG      # 8192 rows per group
CH = 512           # chunk columns (one PSUM bank of fp32)
NSLOT = GR // CH   # 16 chunks per group
HB = BC // 2       # 16384 rows per half
HQ = HB // 128     # 128: per-half batch-major free width
# x_tr column-slice sizes: small first slices for an early compute start
XSLICES = (512, 1536, 2048, 2048, 2048)
OBS_X, OBS_Y, RAD = 4.0, 6.0, 1.5
PI = float(np.pi)

N_WARMUP_MM = 36


def _build_program(std4, mean4, split_waits=True, reps=1):
    nc = bass.Bass()

    x_bm = nc.dram_tensor("x_bm", [BC, NF], F32, kind="ExternalInput")
    x_tr = nc.dram_tensor("x_tr", [32, GR], BF16, kind="ExternalInput")
    w1g = nc.dram_tensor("w1g", [128, 128], BF16, kind="ExternalInput")
    wmw = nc.dram_tensor("wmw", [128, 288], BF16, kind="ExternalInput")
    bias3 = nc.dram_tensor("bias3", [128, 3], F32, kind="ExternalInput")
    u_out = nc.dram_tensor("u", [BC, 2], F32, kind="ExternalOutput")

    with tile.TileContext(nc) as tc:
        from contextlib import ExitStack

        with ExitStack() as ctx:
            _body(ctx, tc, x_bm, x_tr, w1g, wmw, bias3, u_out,
                  std4, mean4, reps)
    if split_waits:
        _split_multi_waits(nc)
    return nc


def _split_multi_waits(nc):
    """walrus (this build) accepts at most one sync-wait per instruction;
    merge same-semaphore waits to their max threshold, then hoist any
    remaining extra waits onto standalone same-engine EventSemaphore ops."""
    for blk in nc.main_func.blocks:
        out = []
        for ins in blk.instructions:
            si = ins.sync_info
            waits = list(si.on_wait) if si is not None else []
            if len(waits) > 1:
                merged = {}
                for w in waits:
                    key = (w.sync_type, w.id)
                    prev = merged.get(key)
                    if (prev is None or (w.wait_value or 0) >
                            (prev.wait_value or 0)):
                        merged[key] = w
                waits = list(merged.values())
                if len(waits) == 1:
                    ins.sync_info = type(si)(on_wait=waits,
                                             on_update=list(si.on_update))
            if len(waits) > 1:
                for k, w in enumerate(waits[:-1]):
                    ev = mybir.InstEventSemaphore(
                        name=f"{ins.name}w{k}", ins=[], outs=[])
                    ev.engine = ins.engine
                    ev.sync_info = type(si)(on_wait=[w], on_update=[])
                    out.append(ev)
                ins.sync_info = type(si)(on_wait=[waits[-1]],
                                         on_update=list(si.on_update))
            out.append(ins)
        blk.instructions = out


def _relu_copy(nc, eng, dst, src, bias):
    """dst = relu(src + bias), PSUM->SBUF, on the given engine."""
    if eng is nc.scalar:
        nc.scalar.activation(dst, src, AF.Relu, bias=bias, scale=1.0)
    else:
        eng.tensor_scalar(dst, src, bias, 0.0, ALU.add, ALU.max)


def _body(ctx, tc, x_bm, x_tr, w1g, wmw, bias3, u_out,
          std4, mean4, reps):
    nc = tc.nc

    const = ctx.enter_context(tc.tile_pool(name="const", bufs=1))
    xtp = ctx.enter_context(tc.tile_pool(name="xtp", bufs=1))
    hp = ctx.enter_context(tc.tile_pool(name="hp", bufs=3))
    mp = ctx.enter_context(tc.tile_pool(name="mp", bufs=4))
    hs = ctx.enter_context(tc.tile_pool(name="hs", bufs=1))
    qp = ctx.enter_context(tc.tile_pool(name="qp", bufs=1))
    ps_h = ctx.enter_context(tc.tile_pool(name="ps_h", bufs=2, space="PSUM"))
    ps_m = ctx.enter_context(tc.tile_pool(name="ps_m", bufs=3, space="PSUM"))
    ps_hd = ctx.enter_context(tc.tile_pool(name="ps_hd", bufs=1, space="PSUM"))

    # ---- input + weight DMAs, ordered for earliest compute start ----
    w1g_sb = const.tile([128, 128], BF16)
    wmw_sb = const.tile([128, 288], BF16)
    bias3_sb = const.tile([128, 3], F32)
    # feature f of group g on partition 32g+f (32-aligned operand bases)
    xt_sb = xtp.tile([128, GR], BF16, name="xt_sb", tag="xt_sb")
    # batch-major x for the dCBF math: col h*1024 + c*8 + f is feature f
    # of row half*HB + p*128 + c
    xh_sb = xtp.tile([128, 2 * HQ * NF], F32, name="xh_sb", tag="xh_sb")

    # sync (HWDGE, earliest): w1g (gates L1+warmups), bias3 (gates the
    # first h-drain), xt slice 0, then wmw (gates L2)
    nc.sync.dma_start(out=w1g_sb, in_=w1g[:, :])
    nc.sync.dma_start(out=bias3_sb, in_=bias3[:, :])
    c0 = 0
    for si, w in enumerate(XSLICES):
        eng = nc.sync if si % 2 == 0 else nc.gpsimd
        for g in range(NG):
            eng.dma_start(out=xt_sb[32 * g:32 * g + NF, c0:c0 + w],
                          in_=x_tr[NF * g:NF * (g + 1), c0:c0 + w])
        if si == 0:
            nc.sync.dma_start(out=wmw_sb, in_=wmw[:, :])
        c0 += w
    # the scalar queue carries only x_bm so the QP x tiles land early
    for h in range(2):
        src3h = x_bm[h * HB:(h + 1) * HB, :].rearrange(
            "(p c) f -> p c f", p=128)
        nc.scalar.dma_start(
            out=xh_sb[:, h * 1024:(h + 1) * 1024]
                .rearrange("p (c f) -> p c f", f=NF),
            in_=src3h,
        )

    wmt_sb = wmw_sb[:, 0:128]
    wz_sb = wmw_sb[:, 128:288]
    b1_sb = bias3_sb[:, 0:1]
    bm_sb = bias3_sb[:, 1:2]
    bh_sb = bias3_sb[:, 2:3]

    for _ in range(reps):
        _body_rep(nc, tc, const, xtp, hp, mp, hs, qp, ps_h, ps_m, ps_hd,
                  u_out, w1g_sb, wmt_sb, wz_sb, b1_sb, bm_sb,
                  bh_sb, xt_sb, xh_sb, std4, mean4)


def _qp_pre(nc, qp, half, xh_sb, std4, mean4):
    """x-only dCBF terms for one half (r_half = p*128 + c). Runs during
    the matmul phase. Returns the tile dict for _qp_post."""
    s0, s1c, s2c, s3 = std4
    m0, m1c, m2c, m3 = mean4

    def t(name):
        nm = f"{name}_{half}"
        return qp.tile([128, HQ], F32, name=nm, tag=nm)

    xs3 = xh_sb[:, half * 1024:(half + 1) * 1024].rearrange(
        "p (c f) -> p c f", f=NF)
    X0, X1, X2, X3 = (xs3[:, :, i] for i in range(4))

    ST, CT, DX, DY, V = t("ST"), t("CT"), t("DX"), t("DY"), t("V")

    def wrapped_sin(out, phase_bias, nm):
        # range-reduce to [-pi, pi] on DVE (Pool TS is ~2us/op; avoid)
        msk = t(f"mk{nm}")
        ph = t(f"ph{nm}")
        ph2 = t(f"p2{nm}")
        if s2c == 1.0 and phase_bias == 0.0:
            ph0 = X2
        else:
            ph0 = t(f"p0{nm}")
            nc.vector.tensor_scalar(ph0, X2, s2c, phase_bias,
                                    ALU.mult, ALU.add)
        nc.vector.tensor_scalar(msk, ph0, PI, None, ALU.is_gt)
        nc.vector.scalar_tensor_tensor(ph, msk, -2.0 * PI, ph0,
                                       ALU.mult, ALU.add)
        nc.vector.tensor_scalar(msk, ph, -PI, None, ALU.is_lt)
        nc.vector.scalar_tensor_tensor(ph2, msk, 2.0 * PI, ph,
                                       ALU.mult, ALU.add)
        nc.scalar.activation(out, ph2, AF.Sin)

    wrapped_sin(ST, m2c, "s")
    wrapped_sin(CT, m2c + PI / 2, "c")
    nc.vector.tensor_scalar(DX, X0, s0, m0 - OBS_X, ALU.mult, ALU.add)
    nc.vector.tensor_scalar(DY, X1, s1c, m1c - OBS_Y, ALU.mult, ALU.add)
    nc.vector.tensor_scalar(V, X3, s3, m3, ALU.mult, ALU.add)

    t1, t2, Aq, t3, t4, Bq = (t("t1"), t("t2"), t("Aq"), t("t3"), t("t4"),
                              t("Bq"))
    nc.gpsimd.tensor_tensor(t1, DX, CT, ALU.mult)
    nc.gpsimd.tensor_tensor(t2, DY, ST, ALU.mult)
    nc.gpsimd.tensor_tensor(Aq, t1, t2, ALU.add)       # A = dx ct + dy st
    nc.gpsimd.tensor_tensor(t3, DX, ST, ALU.mult)
    nc.gpsimd.tensor_tensor(t4, DY, CT, ALU.mult)
    nc.gpsimd.tensor_tensor(Bq, t3, t4, ALU.subtract)  # B = dx st - dy ct

    VB, VA = t("VB"), t("VA")
    nc.gpsimd.tensor_tensor(VB, V, Bq, ALU.mult)       # G1 = 2 VB
    nc.gpsimd.tensor_tensor(VA, V, Aq, ALU.mult)       # bdot = 2 VA

    DX2, DY2, BARp, V2d, VB2, A2 = (t("DX2"), t("DY2"), t("BARp"),
                                    t("V2d"), t("VB2"), t("A2"))
    nc.scalar.activation(DX2, DX, AF.Square)
    nc.scalar.activation(DY2, DY, AF.Square)
    nc.gpsimd.tensor_tensor(BARp, DX2, DY2, ALU.add)   # dx^2 + dy^2
    nc.scalar.activation(V2d, V, AF.Square, scale=float(np.sqrt(2.0)))
    nc.scalar.activation(VB2, VB, AF.Square, scale=2.0)  # G1^2
    nc.scalar.activation(A2, Aq, AF.Square, scale=2.0)   # G2^2

    GG, R = t("GG"), t("R")
    nc.vector.scalar_tensor_tensor(GG, VB2, 1e-12, A2, ALU.add, ALU.add)
    nc.vector.reciprocal(R, GG)
    return dict(Aq=Aq, VB=VB, VA=VA, BARp=BARp, V2d=V2d, R=R, t=t)


def _qp_post(nc, qp, half, pre, headsb, u_out, fast):
    """Head-dependent QP tail for one half. fast=True: DVE main chain
    with GpSimd side ops (for the end-of-kernel tail). fast=False: all
    on GpSimd (STT decomposed into TS+TT; GpSimd has no STT)."""
    t = pre["t"]
    Aq, VB, VA = pre["Aq"], pre["VB"], pre["VA"]
    BARp, V2d, R = pre["BARp"], pre["V2d"], pre["R"]
    if fast:
        ve, va = nc.vector, nc.gpsimd

        def STT(out, in0, sc, in1, op0, op1):
            nc.vector.scalar_tensor_tensor(out, in0, sc, in1, op0, op1)
    else:
        ve = va = nc.gpsimd
        cnt = [0]

        def STT(out, in0, sc, in1, op0, op1):
            tmp = t(f"sx{cnt[0]}")
            cnt[0] += 1
            nc.gpsimd.tensor_scalar(tmp, in0, sc, None, op0)
            nc.gpsimd.tensor_tensor(out, tmp, in1, op1)

    p1n, p2n, sg1, sg2 = t("p1n"), t("p2n"), t("sg1"), t("sg2")
    for v, dst in enumerate([p1n, p2n, sg1, sg2]):
        eng = (nc.sync, nc.gpsimd, nc.scalar, nc.sync)[v]
        eng.dma_start(
            out=dst,
            in_=headsb[32 * v:32 * v + 32, :].rearrange(
                "j (q c) -> j q c", q=4),
        )

    SS, SP, T5p, T4d = t("SS"), t("SP"), t("T5p"), t("T4d")
    ve.tensor_tensor(SS, sg1, sg2, ALU.add)
    va.tensor_tensor(SP, sg1, sg2, ALU.mult)
    STT(T5p, BARp, -RAD * RAD, SP, ALU.add, ALU.mult)
    STT(T4d, SS, 8.0, VA, ALU.mult, ALU.mult)

    T1d, T2d, T3d, q1, q2, NUMn = (t("T1d"), t("T2d"), t("T3d"),
                                   t("q1"), t("q2"), t("NUMn"))
    STT(T1d, VB, 2.0, p1n, ALU.mult, ALU.mult)
    STT(T2d, Aq, 2.0, p2n, ALU.mult, ALU.mult)
    ve.tensor_tensor(T3d, T1d, T2d, ALU.subtract)  # = -Gp
    ve.tensor_tensor(q1, T3d, V2d, ALU.subtract)
    ve.tensor_tensor(q2, q1, T4d, ALU.subtract)
    STT(NUMn, T5p, 16.0, q2, ALU.mult, ALU.subtract)  # = Gp + hcon

    L0, LAM2 = t("L0"), t("LAM2")
    ve.tensor_tensor(L0, NUMn, R, ALU.mult)
    ve.tensor_scalar(LAM2, L0, -2.0, 0.0, ALU.mult, ALU.max)  # 2 lam

    u_bm = qp.tile([128, 2 * HQ], F32, name=f"u_bm_{half}",
                   tag=f"u_bm_{half}")
    ub3 = u_bm[:].rearrange("p (c v) -> p c v", v=2)
    m1t, m2t = t("m1t"), t("m2t")
    ve.tensor_tensor(m1t, LAM2, VB, ALU.mult)
    ve.tensor_tensor(ub3[:, :, 0], p1n, m1t, ALU.subtract)
    va.tensor_tensor(m2t, LAM2, Aq, ALU.mult)
    va.tensor_tensor(ub3[:, :, 1], p2n, m2t, ALU.add)

    uo3 = u_out[half * HB:(half + 1) * HB, :].rearrange(
        "(p c) v -> p c v", p=128)
    nc.gpsimd.dma_start(out=uo3[:, 0:64, :], in_=ub3[:, 0:64, :])
    nc.sync.dma_start(out=uo3[:, 64:128, :], in_=ub3[:, 64:128, :])


def _body_rep(nc, tc, const, xtp, hp, mp, hs, qp, ps_h, ps_m, ps_hd,
              u_out, w1g_sb, wmt_sb, wz_sb, b1_sb, bm_sb, bh_sb,
              xt_sb, xh_sb, std4, mean4):
    # head accumulator: ONE bank shared by both halves; also the
    # PE-warmup dump target
    head_ps = ps_hd.tile([128, CH], F32, name="head", tag="head")

    # PE warmup: dummy matmuls keep the HAM clock ramping while the
    # input DMAs run (overwritten by the real accumulation's start=True)
    for _ in range(N_WARMUP_MM):
        nc.tensor.matmul(head_ps[:, 0:128], w1g_sb[0:8, :],
                         w1g_sb[0:8, 0:128], start=True, stop=True)

    qp_pre = [None, None]
    mi = 0   # m-drain engine rotation counter
    hi = 0   # h-drain second-half rotation counter

    for half in range(2):
        for slot in range(NSLOT):
            # ---- L1: the half's two groups, disjoint 32-row PE strips
            h_ps = ps_h.tile([128, 2 * CH], F32, name="h_ps", tag="h_ps")
            for k in range(2):
                g = 2 * half + k
                nc.tensor.matmul(
                    h_ps[:, k * CH:(k + 1) * CH],
                    w1g_sb[32 * g:32 * g + 8, :],
                    xt_sb[32 * g:32 * g + 8, slot * CH:(slot + 1) * CH],
                    start=True, stop=True,
                    tile_position=(32 * g, 0),
                )
            # GPSIMD cannot read PSUM: drains go to Scalar + Vector only.
            # Alternate: one engine takes h [128,1024], the other takes
            # the two m [128,512] chunks this iter.
            eng_h = nc.scalar if hi % 2 == 0 else nc.vector
            eng_m2 = nc.vector if hi % 2 == 0 else nc.scalar
            hi += 1
            h_sb = hp.tile([128, 2 * CH], BF16, name="h_sb", tag="h_sb")
            _relu_copy(nc, eng_h, h_sb, h_ps, b1_sb)

            for k in range(2):
                jh = k * NSLOT + slot    # chunk index within half
                step = slot * 2 + k      # head accumulation step

                m_ps = ps_m.tile([128, CH], F32, name="m_ps", tag="m_ps")
                nc.tensor.matmul(
                    m_ps, wmt_sb, h_sb[:, k * CH:(k + 1) * CH],
                    start=True, stop=True)
                m_sb = mp.tile([128, CH], BF16, name="m_sb", tag="m_sb")
                mi += 1
                _relu_copy(nc, eng_m2, m_sb, m_ps, bm_sb)

                nc.tensor.matmul(
                    head_ps,
                    wz_sb[:, 31 - jh:159 - jh],
                    m_sb,
                    start=(step == 0), stop=(step == 31),
                )

            if half == 0 and slot == 5:
                qp_pre[0] = _qp_pre(nc, qp, 0, xh_sb, std4, mean4)
            if half == 0 and slot == 15:
                qp_pre[1] = _qp_pre(nc, qp, 1, xh_sb, std4, mean4)
                # prefetch the sigmoid act-table (all sins are traced by
                # now): the 1.3us table load lands here, in Scalar slack,
                # not at the half transition in front of the hsb drain
                warm = qp.tile([1, 1], F32, name="sgwarm", tag="sgwarm")
                nc.scalar.activation(warm, bh_sb[0:1, 0:1], AF.Sigmoid)

        # drain this half's heads to SBUF, then run the QP tail
        hsb = hs.tile([128, CH], F32, name=f"hsb{half}", tag=f"hsb{half}")
        nc.vector.tensor_scalar(hsb[0:64, :], head_ps[0:64, :],
                                -1.0, bh_sb[0:64, :], ALU.mult, ALU.add)
        nc.scalar.activation(hsb[64:128, :], head_ps[64:128, :],
                             AF.Sigmoid, bias=bh_sb[64:128, :], scale=1.0)
        _qp_post(nc, qp, half, qp_pre[half], hsb, u_out, fast=True)


def _host_prepare(inputs):
    """Fold mean/std into L1, build packed weight/bias tensors."""
    import ml_dtypes

    bf16 = ml_dtypes.bfloat16

    x = np.ascontiguousarray(inputs["x"], dtype=np.float32)
    mean = np.asarray(inputs["mean"], dtype=np.float32)
    std = np.asarray(inputs["std"], dtype=np.float32)
    W1 = np.asarray(inputs["W1"], dtype=np.float32)
    b1 = np.asarray(inputs["b1"], dtype=np.float32)
    W21 = np.asarray(inputs["W21"], dtype=np.float32)
    b21 = np.asarray(inputs["b21"], dtype=np.float32)
    W22 = np.asarray(inputs["W22"], dtype=np.float32)
    b22 = np.asarray(inputs["b22"], dtype=np.float32)
    W31 = np.asarray(inputs["W31"], dtype=np.float32)
    b31 = np.asarray(inputs["b31"], dtype=np.float32)
    W32 = np.asarray(inputs["W32"], dtype=np.float32)
    b32 = np.asarray(inputs["b32"], dtype=np.float32)

    W1eff = W1 * std[None, :]                      # [128, 8]
    b1eff = (b1 + W1 @ mean).astype(np.float32)    # [128]
    w1t = np.ascontiguousarray(W1eff.T)            # [8, 128]
    w1g = np.zeros((128, 128), np.float32)
    for g in range(NG):
        w1g[32 * g:32 * g + 8, :] = w1t
    w1g = w1g.astype(bf16)

    Wmid = np.vstack([W21, W22]).astype(np.float32)   # [128, 128]
    wmt = np.ascontiguousarray(Wmid.T)
    bmid = np.concatenate([b21, b22]).astype(np.float32)[:, None]

    Whead = np.zeros((4, 128), np.float32)
    Whead[0:2, 0:64] = W31
    Whead[2:4, 64:128] = W32
    wz = np.zeros((128, 160), np.float32)
    for v in range(4):
        wz[:, 31 + 32 * v] = Whead[v, :]

    bhead = np.zeros((128, 1), np.float32)
    bhead[0:32, 0] = -b31[0]
    bhead[32:64, 0] = -b31[1]
    bhead[64:96, 0] = b32[0]
    bhead[96:128, 0] = b32[1]

    std4 = tuple(float(std[i]) for i in range(4))
    mean4 = tuple(float(mean[i]) for i in range(4))

    wmw = np.ascontiguousarray(
        np.concatenate([wmt, wz], axis=1)).astype(bf16)
    bias3 = np.ascontiguousarray(
        np.concatenate([b1eff[:, None], bmid, bhead], axis=1))

    common = {"w1g": w1g, "wmw": wmw, "bias3": bias3}

    in_maps = []
    for c in range(NCORES):
        xs = x[c * BC:(c + 1) * BC]               # [32768, 8]
        # transposed / grouped layout: row 8g+f = feature f of group g
        xtr = np.ascontiguousarray(
            xs.reshape(NG, GR, NF).transpose(0, 2, 1).reshape(
                32, GR)).astype(bf16)
        in_maps.append({"x_bm": xs, "x_tr": xtr, **common})
    return in_maps, std4, mean4


def kernel(**inputs):
    in_maps, std4, mean4 = _host_prepare(inputs)
    nc = _build_program(std4, mean4)
    last_err = None
    for attempt in range(3):
        try:
            res = run_bass_kernel_spmd(nc, in_maps, list(range(NCORES)))
            break
        except Exception as e:  # transient axon/NRT flakes
            last_err = e
            if attempt == 2:
                raise
            import time

            time.sleep(5)
    u = np.concatenate([res.results[c]["u"] for c in range(NCORES)], axis=0)
    return u.astype(np.float32)


if __name__ == "__main__":
    rng = np.random.default_rng(0)
    demo = {
        "x": rng.standard_normal((B, NF), dtype=np.float32),
        "mean": np.zeros(NF, np.float32),
        "std": np.ones(NF, np.float32),
        "W1": rng.standard_normal((128, NF), dtype=np.float32) * 0.3,
        "b1": rng.standard_normal(128, dtype=np.float32) * 0.3,
        "W21": rng.standard_normal((64, 128), dtype=np.float32) * 0.08,
        "b21": rng.standard_normal(64, dtype=np.float32) * 0.08,
        "W22": rng.standard_normal((64, 128), dtype=np.float32) * 0.08,
        "b22": rng.standard_normal(64, dtype=np.float32) * 0.08,
        "W31": rng.standard_normal((2, 64), dtype=np.float32) * 0.1,
        "b31": rng.standard_normal(2, dtype=np.float32) * 0.1,
        "W32": rng.standard_normal((2, 64), dtype=np.float32) * 0.1,
        "b32": rng.standard_normal(2, dtype=np.float32) * 0.1,
        "sgn": np.int64(1),
    }
    out = kernel(**demo)
    print(out.shape, out.dtype)
